# revision 4
# baseline (speedup 1.0000x reference)
"""DecoderBlock on 8 NeuronCores — token-data-parallel with pair-wise KV
exchange.

Sharding: 2 cores per batch element. Sequence tiles (128 tokens) are
interleaved: core 2b ("even") owns seq tiles {0,2,..,14} of batch b, core
2b+1 ("odd") owns {1,3,..,15}. Every core runs LN1, Q/K/V projections,
causal attention for its own 1024 queries (all 16 heads), out-projection,
LN2 and MLP on its own rows. The only collectives are two pair
AllGathers (groups [[0,1],[2,3],[4,5],[6,7]], Local output — Shared is
unsupported for 2-core groups): K^T right after the K projection (its
transfer hides behind the V projection), then V after the V projection
(hides behind the Q projection). ~4.3MB per rank total, vs 16MB AllGather
+ 2MB AllToAll for the head-tensor-parallel baseline.

Rank-uniform causal structure: for local q-tile j, the needed k-tiles are
block0 (even core's) tiles 0..j and block1 (odd core's) tiles 0..j. Only
the two "diagonal" tiles (i==j of each block) depend on parity:
  even core: dmask0 = tri,   dmask1 = zeros
  odd  core: dmask0 = ones,  dmask1 = tri
passed as input tensors, so the instruction stream is identical on all
cores (SPMD-clean).

Attention uses the baseline's proven inner scheme: scoresT[k,q] on PSUM,
exp without max-subtraction (scores bounded ~3.3), probs @ V_aug with a
ones column giving the softmax denominator from the same matmul.
"""

import numpy as np
import ml_dtypes

B, S, D, H, HD = 4, 2048, 1024, 16, 64
R = 8                       # cores
M = (B * S) // R            # 1024 token rows per core
DM = D * 4                  # MLP hidden 4096
NDT = D // 128              # 8 d-tiles
NBU = D // 128              # 8 d'-bundles (2 heads each)
NMO = DM // 128             # 32 mlp-hidden tiles
NT = M // 128               # 8 row-tiles per core
EPS = 1e-5
BF = ml_dtypes.bfloat16
KVC = H * 65                # 1040 va columns (64 V + 1 ones per head)

_CACHE = {}


def _split_multi_waits(nc):
    """This walrus build allows only ONE sync-wait per instruction
    (setupSyncWait: 'Too many sync wait commands'). Move extra waits onto
    same-engine nops inserted immediately before the instruction."""
    import concourse.mybir as mybir

    for bb in nc.main_func.blocks:
        orig = list(bb.instructions)
        if not any(
            i.sync_info is not None and len(i.sync_info.on_wait) > 1
            for i in orig
        ):
            continue
        new_list = []
        for inst in orig:
            si = inst.sync_info
            if si is not None and len(si.on_wait) > 1:
                waits = list(si.on_wait)
                del si.on_wait[:]
                si.on_wait.append(waits[-1])
                for w in waits[:-1]:
                    n = nc.engines[inst.engine].nop(
                        nofuse=True, hint="wsplit"
                    )
                    cb = nc.cur_bb.bb
                    cb.instructions.remove(n.ins)
                    n.ins.sync_info = mybir.SyncInfo(on_wait=[w], on_update=[])
                    new_list.append(n.ins)
            new_list.append(inst)
        del bb.instructions[:]
        for i in new_list:
            bb.instructions.append(i)


def _build_program(mlp_act=None, no_collectives=False):
    import concourse.bass as bass
    import concourse.mybir as mybir
    import concourse.tile as tile

    f32 = mybir.dt.float32
    bf16 = mybir.dt.bfloat16
    Alu = mybir.AluOpType
    Act = mybir.ActivationFunctionType
    if mlp_act is None:
        mlp_act = Act.Gelu

    nc = bass.Bass("TRN2", target_bir_lowering=False, debug=False, num_devices=R)

    # ---- I/O -------------------------------------------------------------
    x_my = nc.dram_tensor("x_my", [M, D], f32, kind="ExternalInput")
    wq = nc.dram_tensor("wq", [128, NDT, D], bf16, kind="ExternalInput")
    wk = nc.dram_tensor("wk", [128, NDT, D], bf16, kind="ExternalInput")
    wv = nc.dram_tensor("wv", [128, NDT, D], bf16, kind="ExternalInput")
    wo = nc.dram_tensor("wo", [128, NDT, D], bf16, kind="ExternalInput")
    w1 = nc.dram_tensor("w1", [NMO, 128, D], bf16, kind="ExternalInput")
    w2 = nc.dram_tensor("w2", [128, NMO, D], bf16, kind="ExternalInput")
    bq_col = nc.dram_tensor("bq_col", [128, NBU], f32, kind="ExternalInput")
    bk_col = nc.dram_tensor("bk_col", [128, NBU], f32, kind="ExternalInput")
    bv_bc = nc.dram_tensor("bv_bc", [128, KVC], f32, kind="ExternalInput")
    bo_bc = nc.dram_tensor("bo_bc", [128, D], f32, kind="ExternalInput")
    b2_bc = nc.dram_tensor("b2_bc", [128, D], f32, kind="ExternalInput")
    g_col = nc.dram_tensor("g_col", [128, NDT], f32, kind="ExternalInput")
    be_col = nc.dram_tensor("be_col", [128, NDT], f32, kind="ExternalInput")
    b1_col = nc.dram_tensor("b1_col", [128, NMO], f32, kind="ExternalInput")
    dm0 = nc.dram_tensor("dm0", [128, 128], bf16, kind="ExternalInput")
    dm1 = nc.dram_tensor("dm1", [128, 128], bf16, kind="ExternalInput")
    iden = nc.dram_tensor("iden", [128, 128], bf16, kind="ExternalInput")
    out_my = nc.dram_tensor("out_my", [M, D], f32, kind="ExternalOutput")

    C = {}  # shared handles across phase helpers

    def layer_norm_tile(pool, small, xt):
        """xt: [128, D] f32 -> (x-mu)*rstd bf16 (gamma/beta applied
        post-transpose by the caller)."""
        stats = small.tile([128, 2, 6], f32, tag="stats")
        nc.vector.bn_stats(out=stats[:, 0, :], in_=xt[:, 0:512])
        nc.vector.bn_stats(out=stats[:, 1, :], in_=xt[:, 512:1024])
        mv = small.tile([128, 2], f32, tag="mv")
        nc.vector.bn_aggr(out=mv, in_=stats)
        std = small.tile([128, 1], f32, tag="std")
        nc.scalar.activation(
            out=std, in_=mv[:, 1:2], func=Act.Sqrt,
            bias=C["eps_sb"][:, :], scale=1.0,
        )
        rstd = small.tile([128, 1], f32, tag="rstd")
        nc.vector.reciprocal(out=rstd, in_=std)
        nm = small.tile([128, 1], f32, tag="nm")
        nc.vector.tensor_scalar(
            out=nm, in0=mv[:, 0:1],
            scalar1=rstd, scalar2=-1.0,
            op0=Alu.mult, op1=Alu.mult,
        )
        hpre = pool.tile([128, D], bf16, tag="hpre")
        nc.scalar.activation(
            out=hpre, in_=xt, func=Act.Identity,
            bias=nm[:, :], scale=rstd[:, :],
        )
        return hpre

    def phase_ln1(tc, after_first_tile=None):
        hT_sb = C["hT_sb"]
        with (
            tc.tile_pool(name="p1", bufs=3) as p1,
            tc.tile_pool(name="p1s", bufs=4) as p1s,
            tc.tile_pool(name="p1ps", bufs=2, space="PSUM") as p1ps,
        ):
            for t in range(NT):
                xt = p1.tile([128, D], f32, tag="xt")
                nc.sync.dma_start(
                    out=xt, in_=x_my[t * 128:(t + 1) * 128, :]
                )
                if t == 1 and after_first_tile is not None:
                    after_first_tile()
                hpre = layer_norm_tile(p1, p1s, xt)
                for dt in range(NDT):
                    pt = p1ps.tile([128, 128], bf16, tag="pt")
                    nc.tensor.transpose(
                        pt, hpre[:, dt * 128:(dt + 1) * 128], C["iden_sb"]
                    )
                    nc.vector.tensor_scalar(
                        out=hT_sb[:, dt, t * 128:(t + 1) * 128],
                        in0=pt,
                        scalar1=C["g_sb"][:, dt:dt + 1],
                        scalar2=C["be_sb"][:, dt:dt + 1],
                        op0=Alu.mult, op1=Alu.add,
                    )

    def phase_kv_ag_q(tc):
        """K/V projections on own rows, export to kv_my, pair AllGather,
        then Q projection (overlaps the AllGather wait)."""
        hT_sb, QT = C["hT_sb"], C["QT"]
        kt_my, kt_all = C["kt_my"], C["kt_all"]
        va_my, va_all = C["va_my"], C["va_all"]
        wk_sb, wv_sb, wq_sb = C["wk_sb"], C["wv_sb"], C["wq_sb"]
        with (
            tc.tile_pool(name="kvst", bufs=1) as kvst,
            tc.tile_pool(name="psP", bufs=4, space="PSUM") as psP,
        ):
            kt_st = kvst.tile([128, NBU, M], bf16, tag="kt_st")
            va_st = kvst.tile([128, NT, KVC], bf16, tag="va_st")
            nc.vector.memset(va_st, 1.0)

            # K^T: [d' bundle 128, own tokens]
            for bu in range(NBU):
                for sl in range(M // 512):
                    ps = psP.tile([128, 512], f32, tag="psP")
                    for dt in range(NDT):
                        nc.tensor.matmul(
                            ps,
                            lhsT=wk_sb[:, dt, bu * 128:(bu + 1) * 128],
                            rhs=hT_sb[:, dt, sl * 512:(sl + 1) * 512],
                            start=(dt == 0), stop=(dt == NDT - 1),
                        )
                    nc.vector.tensor_scalar_add(
                        out=kt_st[:, bu, sl * 512:(sl + 1) * 512],
                        in0=ps, scalar1=C["bk_sb"][:, bu:bu + 1],
                    )
            nc.sync.dma_start(
                out=kt_my[:, :].rearrange("(bu p) t -> p bu t", p=128),
                in_=kt_st,
            )
            if no_collectives:
                nc.sync.dma_start(out=kt_all[0:M, :], in_=kt_my[:, :])
                nc.sync.dma_start(out=kt_all[M:2 * M, :], in_=kt_my[:, :])
            else:
                nc.gpsimd.collective_compute(
                    "AllGather",
                    mybir.AluOpType.bypass,
                    replica_groups=[[0, 1], [2, 3], [4, 5], [6, 7]],
                    ins=[kt_my.opt()],
                    outs=[kt_all.opt()],
                )
            # V: [own tok 128, d'] + bias into 65-slot va layout
            for t in range(NT):
                for sl in range(2):
                    ps = psP.tile([128, 512], f32, tag="psP")
                    for dt in range(NDT):
                        nc.tensor.matmul(
                            ps,
                            lhsT=hT_sb[:, dt, t * 128:(t + 1) * 128],
                            rhs=wv_sb[:, dt, sl * 512:(sl + 1) * 512],
                            start=(dt == 0), stop=(dt == NDT - 1),
                        )
                    h0 = sl * 8
                    nc.vector.tensor_tensor(
                        out=va_st[:, t, :].rearrange(
                            "p (h c) -> p h c", c=65)[:, h0:h0 + 8, 0:64],
                        in0=ps.rearrange("p (h c) -> p h c", c=64),
                        in1=C["bvbc_sb"][:, :].rearrange(
                            "p (h c) -> p h c", c=65)[:, h0:h0 + 8, 0:64],
                        op=Alu.add,
                    )
            nc.sync.dma_start(
                out=va_my[:, :].rearrange("(tt p) c -> p tt c", p=128),
                in_=va_st,
            )
            if no_collectives:
                nc.sync.dma_start(out=va_all[0:M, :], in_=va_my[:, :])
                nc.sync.dma_start(out=va_all[M:2 * M, :], in_=va_my[:, :])
            else:
                nc.gpsimd.collective_compute(
                    "AllGather",
                    mybir.AluOpType.bypass,
                    replica_groups=[[0, 1], [2, 3], [4, 5], [6, 7]],
                    ins=[va_my.opt()],
                    outs=[va_all.opt()],
                )

            # Q^T on own rows (overlaps the AllGather wait)
            for bu in range(NBU):
                for sl in range(M // 512):
                    ps = psP.tile([128, 512], f32, tag="psP")
                    for dt in range(NDT):
                        nc.tensor.matmul(
                            ps,
                            lhsT=wq_sb[:, dt, bu * 128:(bu + 1) * 128],
                            rhs=hT_sb[:, dt, sl * 512:(sl + 1) * 512],
                            start=(dt == 0), stop=(dt == NDT - 1),
                        )
                    nc.vector.tensor_scalar_add(
                        out=QT[:, bu, sl * 512:(sl + 1) * 512],
                        in0=ps, scalar1=C["bq_sb"][:, bu:bu + 1],
                    )

    def attn_head_group(h, J, KT, va, psA, psO, aex, asm):
        QT, aosb = C["QT"], C["aosb"]
        bu, half = h // 2, h % 2
        hofs = half * 64
        dmm = (C["dm0_sb"], C["dm1_sb"])
        pso = [
            psO.tile([128, 65], mybir.dt.float32, tag="pso",
                     name=f"pso_{h}_{J}_{i}")
            for i in range(4)
        ]
        n_sk = 4 * J + 4
        for be_ in range(2):
            for i0 in range(0, n_sk, 2):
                pss = psA.tile([128, 1024], mybir.dt.float32, tag="psA")
                for jj in range(2):
                    nc.tensor.matmul(
                        pss[:, jj * 512:(jj + 1) * 512],
                        lhsT=KT[hofs:hofs + 64, bu,
                                be_ * M + (i0 + jj) * 128:
                                be_ * M + (i0 + jj + 1) * 128],
                        rhs=QT[hofs:hofs + 64, bu, J * 512:(J + 1) * 512],
                        start=True, stop=True,
                    )
                ex = aex.tile([128, 1024], mybir.dt.bfloat16, tag="ex")
                nc.scalar.activation(out=ex, in_=pss, func=Act.Exp)
                for jj in range(2):
                    i = i0 + jj
                    for sq in range(4):
                        j = 4 * J + sq
                        if i > j:
                            continue
                        exs = ex[:, jj * 512 + sq * 128:
                                 jj * 512 + (sq + 1) * 128]
                        if i == j:
                            nc.vector.tensor_mul(
                                out=exs, in0=exs, in1=dmm[be_],
                            )
                        nc.tensor.matmul(
                            pso[sq],
                            lhsT=exs,
                            rhs=va[:, be_ * NT + i, h * 65:(h + 1) * 65],
                            start=(be_ == 0 and i == 0),
                            stop=(be_ == 1 and i == j),
                        )
        for sq in range(4):
            j = 4 * J + sq
            rec = asm.tile([128, 1], mybir.dt.float32, tag="rec")
            nc.vector.reciprocal(out=rec, in_=pso[sq][:, 64:65])
            nc.vector.tensor_scalar_mul(
                out=aosb[:, j, h * 64:(h + 1) * 64],
                in0=pso[sq][:, 0:64],
                scalar1=rec,
            )

    def phase_attention(tc):
        kt_all, va_all = C["kt_all"], C["va_all"]
        with (
            tc.tile_pool(name="kva", bufs=1) as kva,
            tc.tile_pool(name="aex", bufs=5) as aex,
            tc.tile_pool(name="asm", bufs=8) as asm,
            tc.tile_pool(name="psA", bufs=2, space="PSUM") as psA,
            tc.tile_pool(name="psO", bufs=4, space="PSUM") as psO,  # [128,4,65] = 1 bank each
        ):
            # cols: [block0 tokens | block1 tokens]
            KT = kva.tile([128, NBU, 2 * M], mybir.dt.bfloat16, tag="KT")
            va = kva.tile([128, 2 * NT, KVC], mybir.dt.bfloat16, tag="va")
            for be_ in range(2):
                nc.sync.dma_start(
                    out=KT[:, :, be_ * M:(be_ + 1) * M],
                    in_=kt_all[be_ * M:(be_ + 1) * M, :].rearrange(
                        "(bu p) t -> p bu t", p=128),
                )
                nc.sync.dma_start(
                    out=va[:, be_ * NT:(be_ + 1) * NT, :],
                    in_=va_all[be_ * M:(be_ + 1) * M, :].rearrange(
                        "(tt p) c -> p tt c", p=128),
                )
            for h in range(H):
                for J in range(2):
                    attn_head_group(h, J, KT, va, psA, psO, aex, asm)

    def phase_proj_ln2(tc):
        """aoT transposes, out-projection + residual, LN2 -> h2T."""
        aosb, h2T = C["aosb"], C["h2T"]
        with (
            tc.tile_pool(name="p5a", bufs=1) as p5a,
            tc.tile_pool(name="p5", bufs=2) as p5,
            tc.tile_pool(name="p5s", bufs=4) as p5s,
            tc.tile_pool(name="psB1", bufs=3, space="PSUM") as psB,
            tc.tile_pool(name="psT", bufs=2, space="PSUM") as psT,
        ):
            aoT = p5a.tile([128, NDT, M], mybir.dt.bfloat16, tag="aoT")
            wo_sb = p5a.tile([128, NDT, D], mybir.dt.bfloat16, tag="wo")
            nc.sync.dma_start(out=wo_sb, in_=wo[:, :, :])
            bobc_sb = p5a.tile([128, D], mybir.dt.float32, tag="bo")
            nc.sync.dma_start(out=bobc_sb, in_=bo_bc[:, :])
            for t in range(NT):
                for dt in range(NDT):
                    pt = psT.tile([128, 128], mybir.dt.bfloat16, tag="pt")
                    nc.tensor.transpose(
                        pt, aosb[:, t, dt * 128:(dt + 1) * 128], C["iden_sb"]
                    )
                    nc.vector.tensor_copy(
                        out=aoT[:, dt, t * 128:(t + 1) * 128], in_=pt,
                    )
            for t in range(NT):
                xt = p5.tile([128, D], mybir.dt.float32, tag="xt5")
                nc.sync.dma_start(
                    out=xt, in_=x_my[t * 128:(t + 1) * 128, :]
                )
                x2t = p5.tile([128, D], mybir.dt.float32, tag="x2t")
                for sl in range(2):
                    psp = psB.tile([128, 512], mybir.dt.float32, tag="psB")
                    for dt in range(NDT):
                        nc.tensor.matmul(
                            psp,
                            lhsT=aoT[:, dt, t * 128:(t + 1) * 128],
                            rhs=wo_sb[:, dt, sl * 512:(sl + 1) * 512],
                            start=(dt == 0), stop=(dt == NDT - 1),
                        )
                    nc.vector.tensor_tensor(
                        out=x2t[:, sl * 512:(sl + 1) * 512],
                        in0=psp,
                        in1=xt[:, sl * 512:(sl + 1) * 512],
                        op=Alu.add,
                    )
                nc.vector.tensor_tensor(
                    out=x2t, in0=x2t, in1=bobc_sb, op=Alu.add,
                )
                nc.sync.dma_start(
                    out=C["x2_dram"][t * 128:(t + 1) * 128, :], in_=x2t
                )
                h2pre = layer_norm_tile(p5, p5s, x2t)
                for dt in range(NDT):
                    pt = psT.tile([128, 128], mybir.dt.bfloat16, tag="pt")
                    nc.tensor.transpose(
                        pt, h2pre[:, dt * 128:(dt + 1) * 128], C["iden_sb"]
                    )
                    nc.vector.tensor_scalar(
                        out=h2T[t // 4][:, dt,
                                        (t % 4) * 128:(t % 4 + 1) * 128],
                        in0=pt,
                        scalar1=C["g_sb"][:, dt:dt + 1],
                        scalar2=C["be_sb"][:, dt:dt + 1],
                        op0=Alu.mult, op1=Alu.add,
                    )

    def phase_mlp(tc):
        h2T = C["h2T"]
        with (
            tc.tile_pool(name="p5m", bufs=1) as p5m,
            tc.tile_pool(name="w1p", bufs=3) as w1p,
            tc.tile_pool(name="p5t", bufs=2) as p5t,
            tc.tile_pool(name="psB2", bufs=3, space="PSUM") as psB,
        ):
            m_sb = p5m.tile([128, NMO, M], mybir.dt.bfloat16, tag="m")
            w2_sb = p5m.tile([128, NMO, D], mybir.dt.bfloat16, tag="w2")
            # gpsimd DMA queue: don't block the w1 tile stream on nc.sync
            nc.gpsimd.dma_start(out=w2_sb, in_=w2[:, :, :])
            b2bc_sb = p5m.tile([128, D], mybir.dt.float32, tag="b2bc")
            nc.gpsimd.dma_start(out=b2bc_sb, in_=b2_bc[:, :])
            for mo in range(NMO):
                w1t = w1p.tile([128, D], mybir.dt.bfloat16, tag="w1t")
                nc.scalar.dma_start(out=w1t, in_=w1[mo, :, :])
                for sl in range(2):
                    psm = psB.tile([128, 512], mybir.dt.float32, tag="psB")
                    for dt in range(NDT):
                        nc.tensor.matmul(
                            psm,
                            lhsT=w1t[:, dt * 128:(dt + 1) * 128],
                            rhs=h2T[sl][:, dt, :],
                            start=(dt == 0), stop=(dt == NDT - 1),
                        )
                    nc.scalar.activation(
                        out=m_sb[:, mo, sl * 512:(sl + 1) * 512],
                        in_=psm, func=mlp_act,
                        bias=C["b1_sb"][:, mo:mo + 1], scale=1.0,
                    )
            for t in range(NT):
                for sl in range(2):
                    psy = psB.tile([128, 512], mybir.dt.float32, tag="psB")
                    for mo in range(NMO):
                        nc.tensor.matmul(
                            psy,
                            lhsT=m_sb[:, mo, t * 128:(t + 1) * 128],
                            rhs=w2_sb[:, mo, sl * 512:(sl + 1) * 512],
                            start=(mo == 0), stop=(mo == NMO - 1),
                        )
                    xb = p5t.tile([128, 512], mybir.dt.float32, tag="xb")
                    nc.sync.dma_start(
                        out=xb,
                        in_=C["x2_dram"][t * 128:(t + 1) * 128,
                                         sl * 512:(sl + 1) * 512],
                    )
                    ot = p5t.tile([128, 512], mybir.dt.float32, tag="ot")
                    nc.vector.tensor_tensor(
                        out=ot, in0=psy, in1=xb, op=Alu.add,
                    )
                    nc.vector.tensor_tensor(
                        out=ot, in0=ot,
                        in1=b2bc_sb[:, sl * 512:(sl + 1) * 512],
                        op=Alu.add,
                    )
                    nc.sync.dma_start(
                        out=out_my[t * 128:(t + 1) * 128,
                                   sl * 512:(sl + 1) * 512],
                        in_=ot,
                    )

    with tile.TileContext(nc) as tc:
        with tc.tile_pool(name="dram", bufs=1, space="DRAM") as dram:
            C["kt_my"] = dram.tile([M, D], mybir.dt.bfloat16,
                                   tag="kt_my", name="kt_my")
            C["kt_all"] = dram.tile([2 * M, D], mybir.dt.bfloat16,
                                    tag="kt_all", name="kt_all")
            C["va_my"] = dram.tile([M, KVC], mybir.dt.bfloat16,
                                   tag="va_my", name="va_my")
            C["va_all"] = dram.tile([2 * M, KVC], mybir.dt.bfloat16,
                                    tag="va_all", name="va_all")
            C["x2_dram"] = dram.tile([M, D], f32, tag="x2_dram", name="x2_dram")

            with tc.tile_pool(name="consts", bufs=1) as consts:
                for nm_, src, shp in (
                    ("iden_sb", iden, [128, 128]),
                    ("dm0_sb", dm0, [128, 128]),
                    ("dm1_sb", dm1, [128, 128]),
                ):
                    t_ = consts.tile(shp, bf16, tag=nm_, name=nm_)
                    nc.sync.dma_start(out=t_, in_=src[:, :])
                    C[nm_] = t_
                for nm_, src, shp in (
                    ("g_sb", g_col, [128, NDT]),
                    ("be_sb", be_col, [128, NDT]),
                    ("bq_sb", bq_col, [128, NBU]),
                    ("bk_sb", bk_col, [128, NBU]),
                    ("bvbc_sb", bv_bc, [128, KVC]),
                    ("b1_sb", b1_col, [128, NMO]),
                ):
                    t_ = consts.tile(shp, f32, tag=nm_, name=nm_)
                    nc.sync.dma_start(out=t_, in_=src[:, :])
                    C[nm_] = t_
                eps_sb = consts.tile([128, 1], f32, tag="eps")
                nc.vector.memset(eps_sb, EPS)
                C["eps_sb"] = eps_sb

                with tc.tile_pool(name="p5big", bufs=1) as p5big:
                    C["h2T"] = [
                        p5big.tile([128, NDT, 512], bf16, tag=f"h2T{sl}",
                                   name=f"h2T{sl}")
                        for sl in range(2)
                    ]
                    with tc.tile_pool(name="aob", bufs=1) as aob:
                        C["aosb"] = aob.tile([128, NT, D], bf16, tag="aosb", name="aosb")
                        with tc.tile_pool(name="qt", bufs=1) as qtp:
                            C["QT"] = qtp.tile([128, NBU, M], bf16, tag="QT", name="QT")
                            with tc.tile_pool(name="p1h", bufs=1) as p1h:
                                C["hT_sb"] = p1h.tile([128, NDT, M], bf16,
                                                      tag="hT", name="hT")
                                with tc.tile_pool(name="qkw", bufs=1) as qkw:
                                    def load_qkv_weights():
                                        for nm_, src in (("wk_sb", wk),
                                                         ("wv_sb", wv),
                                                         ("wq_sb", wq)):
                                            t_ = qkw.tile(
                                                [128, NDT, D], bf16,
                                                tag=nm_, name=nm_)
                                            nc.gpsimd.dma_start(
                                                out=t_, in_=src[:, :, :])
                                            C[nm_] = t_
                                    phase_ln1(
                                        tc,
                                        after_first_tile=load_qkv_weights)
                                    phase_kv_ag_q(tc)
                            phase_attention(tc)
                        phase_proj_ln2(tc)
                    phase_mlp(tc)
    _split_multi_waits(nc)
    return nc


def _row_perm(core):
    """Global token-row indices owned by `core`, in local order."""
    b, p = core // 2, core % 2
    rows = []
    for i in range(NT):
        t_seq = 2 * i + p
        base = b * S + t_seq * 128
        rows.extend(range(base, base + 128))
    return np.asarray(rows)


def _prep_inputs(x, Wq, Wk, Wv, bq, bk, bv, Wo, bo, W1, b1, W2, b2, gamma, beta):
    """Shard + cast host-side; returns list of per-core input dicts."""
    xf = np.ascontiguousarray(x.reshape(B * S, D), dtype=np.float32)
    tri = np.triu(np.ones((128, 128), np.float32)).astype(BF)
    ones = np.ones((128, 128), np.float32).astype(BF)
    zeros = np.zeros((128, 128), np.float32).astype(BF)
    iden = np.eye(128, dtype=np.float32).astype(BF)
    g_col = np.ascontiguousarray(gamma.reshape(NDT, 128).T, dtype=np.float32)
    be_col = np.ascontiguousarray(beta.reshape(NDT, 128).T, dtype=np.float32)
    b1_col = np.ascontiguousarray(b1.reshape(NMO, 128).T, dtype=np.float32)
    bo_bc = np.ascontiguousarray(
        np.broadcast_to(bo.astype(np.float32), (128, D)))
    b2_bc = np.ascontiguousarray(
        np.broadcast_to(b2.astype(np.float32), (128, D)))

    def wt_t(w2d):  # [D, D'] -> [128, NDT, D'] (d-tile partition-major)
        dp = w2d.shape[1]
        return np.ascontiguousarray(
            np.asarray(w2d, np.float32).reshape(NDT, 128, dp)
            .transpose(1, 0, 2)).astype(BF)

    Wq_all = np.concatenate([Wq[h] for h in range(H)], axis=1) * 0.125
    Wk_all = np.concatenate([Wk[h] for h in range(H)], axis=1)
    Wv_all = np.concatenate([Wv[h] for h in range(H)], axis=1)
    wq_t, wk_t, wv_t = wt_t(Wq_all), wt_t(Wk_all), wt_t(Wv_all)
    wo_t = wt_t(Wo)
    w1_t = np.ascontiguousarray(
        W1.reshape(NDT, 128, NMO, 128).transpose(2, 1, 0, 3).reshape(
            NMO, 128, D)).astype(BF)
    w2_t = np.ascontiguousarray(
        W2.reshape(NMO, 128, D).transpose(1, 0, 2)).astype(BF)

    bq_col = np.ascontiguousarray(
        (np.asarray(bq, np.float32) * 0.125).reshape(NBU, 128).T)
    bk_col = np.ascontiguousarray(
        np.asarray(bk, np.float32).reshape(NBU, 128).T)
    bv_bc = np.zeros((128, KVC), np.float32)
    for h in range(H):
        bv_bc[:, h * 65:h * 65 + 64] = np.asarray(bv[h], np.float32)

    common = {
        "wq": wq_t, "wk": wk_t, "wv": wv_t,
        "wo": wo_t, "w1": w1_t, "w2": w2_t,
        "bq_col": bq_col, "bk_col": bk_col, "bv_bc": bv_bc,
        "bo_bc": bo_bc, "b2_bc": b2_bc,
        "g_col": g_col, "be_col": be_col, "b1_col": b1_col,
        "iden": iden,
    }
    in_maps = []
    for r in range(R):
        p = r % 2
        in_maps.append(dict(
            common,
            x_my=np.ascontiguousarray(xf[_row_perm(r)]),
            dm0=(tri if p == 0 else ones),
            dm1=(zeros if p == 0 else tri),
        ))
    return in_maps


def kernel(**inputs):
    inputs = {k: np.asarray(v) for k, v in inputs.items()}
    in_maps = _prep_inputs(**inputs)
    if "nc" not in _CACHE:
        _CACHE["nc"] = _build_program()
    from concourse.bass_utils import run_bass_kernel_spmd
    res = run_bass_kernel_spmd(_CACHE["nc"], in_maps, list(range(R)))
    _CACHE["last_res"] = res
    out = np.empty((B * S, D), np.float32)
    for r in range(R):
        out[_row_perm(r)] = res.results[r]["out_my"]
    return np.ascontiguousarray(out.reshape(B, S, D), dtype=np.float32)
